# revision 7
# baseline (speedup 1.0000x reference)
"""CategorySpecificLinear on 8 TRN2 NeuronCores.

out[b, t, h] = sum_i x[b, t, i] * W[cat_ids[b], i, h] + bias[cat_ids[b], h]

Strategy: data-parallel over the batch in bf16. Samples are sorted by
category on the host and dealt to cores in groups of 8. Within a core the
8 samples are interleaved into two "lanes" (even/odd positions) so that
samples sharing a category sit 2 apart and can reuse one of two resident
SBUF weight buffers (wbuf[s % 2]); a per-sample flag makes the 4 MiB
weight load conditional (tc.If), cutting HBM weight traffic to ~one load
per distinct category. x / W are cast to bf16 and pre-arranged on the
host so every DMA is a straight [128, free] copy; matmuls run bf16 at
full PE rate accumulating fp32 in PSUM; outputs are stored bf16 and
upcast on the host.
"""

import os
import sys

import numpy as np

for _p in (
    "/opt/trn_rl_repo",
    os.path.expanduser("~/.axon_site/_ro/trn_rl_repo"),
):
    if os.path.isdir(_p) and _p not in sys.path:
        sys.path.insert(0, _p)

import ml_dtypes  # noqa: E402

import concourse.bass as bass  # noqa: E402
import concourse.mybir as mybir  # noqa: E402
import concourse.tile as tile  # noqa: E402
from concourse import bacc  # noqa: E402
from concourse.bass_utils import run_bass_kernel_spmd  # noqa: E402

NCORES = 8
B, T, I, H, NCAT = 64, 256, 1024, 2048, 32
S = B // NCORES  # samples per core
KK = I // 128  # K chunks of 128
NN = H // 512  # N chunks of 512
MM = T // 128  # M chunks of 128
F32 = mybir.dt.float32
BF16 = mybir.dt.bfloat16
NPBF16 = ml_dtypes.bfloat16

_cache: dict = {}


def _build(has_bias: bool):
    nc = bacc.Bacc(
        "TRN2", target_bir_lowering=False, debug=False, num_devices=NCORES
    )
    # x pre-arranged on host: [S, 128, KK*T], (s, p, kk*T+t) = x[s, t, kk*128+p]
    x_in = nc.dram_tensor("xr", [S, 128, KK * T], BF16, kind="ExternalInput")
    cats_in = nc.dram_tensor("cats", [1, S], mybir.dt.int32, kind="ExternalInput")
    flags_in = nc.dram_tensor("flags", [1, S], mybir.dt.int32, kind="ExternalInput")
    reps_in = nc.dram_tensor("reps", [1, 1], mybir.dt.int32, kind="ExternalInput")
    # W pre-arranged on host: [NCAT, 128, KK*H], (c, p, kk*H+h) = W[c, kk*128+p, h]
    W_in = nc.dram_tensor("Wr", [NCAT, 128, KK * H], BF16, kind="ExternalInput")
    if has_bias:
        b_in = nc.dram_tensor("b", [NCAT, H], BF16, kind="ExternalInput")
    out_o = nc.dram_tensor("out", [S, T, H], BF16, kind="ExternalOutput")

    SP = mybir.EngineType.SP
    ACT = mybir.EngineType.Activation

    with tile.TileContext(nc) as tc:
        with (
            tc.tile_pool(name="const", bufs=1) as cpool,
            tc.tile_pool(name="data", bufs=2) as dpool,
            tc.tile_pool(name="mmps", bufs=2, space="PSUM") as mmpool,
        ):
            cats_sb = cpool.tile([1, S], mybir.dt.int32)
            nc.sync.dma_start(cats_sb[:], cats_in[:])
            flags_sb = cpool.tile([1, S], mybir.dt.int32)
            nc.sync.dma_start(flags_sb[:], flags_in[:])
            reps_sb = cpool.tile([1, 1], mybir.dt.int32)
            nc.sync.dma_start(reps_sb[:], reps_in[:])

            cat_vals = [
                nc.values_load(
                    cats_sb[0:1, s : s + 1],
                    engines=[SP, ACT],
                    min_val=0,
                    max_val=NCAT - 1,
                    skip_runtime_bounds_check=True,
                )
                for s in range(S)
            ]
            flag_vals = [
                nc.values_load(
                    flags_sb[0:1, s : s + 1],
                    engines=[SP, ACT],
                    min_val=0,
                    max_val=1,
                    skip_runtime_bounds_check=True,
                )
                for s in range(S)
            ]
            reps_val = nc.values_load(
                reps_sb[0:1, 0:1],
                min_val=1,
                max_val=1 << 20,
                skip_runtime_bounds_check=True,
            )

            wbufs = [cpool.tile([128, KK * H], BF16, name=f"wbuf{i}") for i in range(2)]
            if has_bias:
                bbufs = [cpool.tile([128, H], BF16, name=f"bbuf{i}") for i in range(2)]

            with tc.For_i(0, reps_val, 1):
                for s in range(S):
                    cv = cat_vals[s]
                    wb = wbufs[s % 2]
                    wdma = nc.sync if s % 2 == 0 else nc.scalar
                    with tc.If(flag_vals[s] > 0):
                        wdma.dma_start(
                            wb[:], W_in[bass.ds(cv, 1), :, :].squeeze(0)
                        )
                        if has_bias:
                            wdma.dma_start(
                                bbufs[s % 2][:],
                                b_in[bass.ds(cv, 1), :].to_broadcast((128, H)),
                            )

                    xt = dpool.tile([128, KK * T], BF16, tag="xt")
                    nc.gpsimd.dma_start(xt[:], x_in[s])

                    for m in range(MM):
                        pss = [
                            mmpool.tile(
                                [128, 512], F32, tag=f"ps{n}", name=f"ps{n}"
                            )
                            for n in range(NN)
                        ]
                        # kk outer / n inner: the stationary x-tile is reused
                        # across the 4 N-chunks, so LDWEIGHTS runs once per
                        # (m, kk) instead of once per matmul.
                        for kk in range(KK):
                            for n in range(NN):
                                nc.tensor.matmul(
                                    pss[n][:],
                                    xt[:, kk * T + m * 128 : kk * T + (m + 1) * 128],
                                    wb[:, kk * H + n * 512 : kk * H + (n + 1) * 512],
                                    start=(kk == 0),
                                    stop=(kk == KK - 1),
                                )
                        ot = dpool.tile([128, H], BF16, tag="ot")
                        for n in range(NN):
                            if has_bias:
                                nc.vector.tensor_add(
                                    ot[:, n * 512 : (n + 1) * 512],
                                    pss[n][:],
                                    bbufs[s % 2][:, n * 512 : (n + 1) * 512],
                                )
                            else:
                                nc.vector.tensor_copy(
                                    ot[:, n * 512 : (n + 1) * 512], pss[n][:]
                                )
                        nc.gpsimd.dma_start(
                            out_o[s, m * 128 : (m + 1) * 128, :], ot[:]
                        )

    nc.compile()
    return nc


def _get_nc(has_bias: bool):
    key = ("nc", has_bias)
    if key not in _cache:
        _cache[key] = _build(has_bias)
    return _cache[key]


def plan_order(cat_ids):
    """Global sample permutation + per-sample load flags.

    Sorts samples by category, deals contiguous groups of S to each core,
    then within a core packs the category groups into two 4-slot lanes
    (first-fit decreasing) and interleaves them, so samples of the same
    category appear 2 apart and share wbuf[s % 2]. flag[s]=1 when slot
    s % 2 must be (re)loaded.
    """
    base = np.argsort(cat_ids, kind="stable")
    perm = np.empty(B, dtype=np.int64)
    flags = np.empty(B, dtype=np.int32)
    half = S // 2
    for c in range(NCORES):
        idx = base[c * S : (c + 1) * S]
        cats = cat_ids[idx]
        groups = []  # (cat, [sample indices]) in sorted order
        for i, g in enumerate(idx):
            if groups and cats[i] == groups[-1][0]:
                groups[-1][1].append(g)
            else:
                groups.append((int(cats[i]), [g]))
        lanes = [[], []]
        for cat, members in sorted(groups, key=lambda g: -len(g[1])):
            members = list(members)
            while members:
                li = 0 if len(lanes[0]) <= len(lanes[1]) else 1
                if len(lanes[li]) >= half:
                    li = 1 - li
                take = min(len(members), half - len(lanes[li]))
                lanes[li].extend((cat, m) for m in members[:take])
                members = members[take:]
        for s in range(S):
            cat, g = lanes[s % 2][s // 2]
            perm[c * S + s] = g
            flags[c * S + s] = (
                1 if s < 2 or cat_ids[perm[c * S + s - 2]] != cat else 0
            )
    return perm, flags


def _prep_w(W):
    Wb = W.astype(NPBF16)
    return np.ascontiguousarray(
        Wb.reshape(NCAT, KK, 128, H).transpose(0, 2, 1, 3)
    ).reshape(NCAT, 128, KK * H)


def _make_in_maps(x, cat_ids, W, b, has_bias, perm, flags, reps=1, Wr=None):
    if Wr is None:
        Wr = _prep_w(W)
    in_maps = []
    for c in range(NCORES):
        idx = perm[c * S : (c + 1) * S]
        xs = x[idx].astype(NPBF16)  # [S, T, I]
        xr = np.ascontiguousarray(
            xs.reshape(S, T, KK, 128).transpose(0, 3, 2, 1)
        ).reshape(S, 128, KK * T)
        m = {
            "xr": xr,
            "cats": cat_ids[idx].reshape(1, S),
            "flags": flags[c * S : (c + 1) * S].reshape(1, S),
            "reps": np.full((1, 1), reps, dtype=np.int32),
            "Wr": Wr,
        }
        if has_bias:
            m["b"] = b.astype(NPBF16)
        in_maps.append(m)
    return in_maps


def kernel(x, cat_ids, W, b):
    x = np.ascontiguousarray(np.asarray(x, dtype=np.float32))
    cat_ids = np.asarray(cat_ids, dtype=np.int32)
    W = np.ascontiguousarray(np.asarray(W, dtype=np.float32))
    b = np.asarray(b, dtype=np.float32)
    assert x.shape == (B, T, I) and cat_ids.shape == (B,)
    assert W.shape == (NCAT, I, H) and b.shape == (NCAT, H)

    has_bias = bool(np.any(b))
    nc = _get_nc(has_bias)

    perm, flags = plan_order(cat_ids)
    in_maps = _make_in_maps(x, cat_ids, W, b, has_bias, perm, flags)

    res = run_bass_kernel_spmd(nc, in_maps, list(range(NCORES)))

    out = np.empty((B, T, H), dtype=np.float32)
    for c in range(NCORES):
        idx = perm[c * S : (c + 1) * S]
        out[idx] = np.asarray(res.results[c]["out"]).astype(np.float32)
    return out
